# revision 12
# baseline (speedup 1.0000x reference)
"""GroupedQueryAttention TRN2 Bass kernel, sharded over 8 NeuronCores.

Problem (hardcoded): B=2, T=2048, D=4096, 32 Q heads x 128, 8 KV groups x 128,
RoPE (base 5e5), causal, out = ctx @ Wo.

Sharding: core g owns Q heads 4g..4g+3 (Wq columns 512g:512g+512), KV group g
(Wk/Wv columns 128g:128g+128), and Wo rows 512g:512g+512 (row-parallel).
Each core computes a full-shape partial output; host sums the 8 partials.

v5 (over v4): single pool scope with a unified PSUM ring — no pool-close
barrier between projection and attention phases, so the attention seeds
overlap the last projection tile's RoPE tail; softmax denominator reduction
via an all-ones [128,128] stationary matmul that reduces AND broadcasts in
one op (drops the gpsimd broadcast from the critical path); output-projection
evictions moved off the exp-laden ACT queue (DVE/gpsimd rotate); Wo reuses
Wq's SBUF ring slot; startup DMAs issued from four engine queues in
parallel; drain-phase output DMAs split across two queues.
"""
import sys
import numpy as np

for _p in ("/opt/trn_rl_repo", "/root/.axon_site", "/root/.axon_site/_ro/trn_rl_repo"):
    if _p not in sys.path:
        sys.path.append(_p)

from contextlib import ExitStack

import ml_dtypes

import concourse.bass as bass
import concourse.tile as tile
from concourse import bacc, mybir
from concourse.bass_utils import run_bass_kernel_spmd
from concourse.masks import make_identity

B, T, D = 2, 2048, 4096
NH, NKV, DH = 32, 8, 128
HPC = NH // 8          # 4 q heads per core
FPC = HPC * DH         # 512 q features per core
ROPE_BASE = 500000.0
NT = B * T             # 4096 tokens
f32 = mybir.dt.float32
bf16 = mybir.dt.bfloat16
EXP_SCALE = 1.0 / float(np.sqrt(DH))
BF = ml_dtypes.bfloat16

_NC_CACHE = {}


def _build_program():
    nc = bacc.Bacc("TRN2", target_bir_lowering=False, debug=False)

    xT = nc.dram_tensor("xT", [D, NT], bf16, kind="ExternalInput").ap()
    wq = nc.dram_tensor("wq", [D, FPC], bf16, kind="ExternalInput").ap()
    wk = nc.dram_tensor("wk", [D, DH], bf16, kind="ExternalInput").ap()
    wv = nc.dram_tensor("wv", [D, DH], bf16, kind="ExternalInput").ap()
    wo = nc.dram_tensor("wo", [FPC, D], bf16, kind="ExternalInput").ap()
    ropeA = nc.dram_tensor("ropeA", [128, T], bf16, kind="ExternalInput").ap()
    ropeB = nc.dram_tensor("ropeB", [128, T], bf16, kind="ExternalInput").ap()
    trim = nc.dram_tensor("trim", [128, 128], bf16, kind="ExternalInput").ap()
    # output stored as contiguous [128,512] blocks: gi = m*8 + n_
    outp = nc.dram_tensor("outp", [(NT // 128) * (D // 512), 128, 512], bf16,
                          kind="ExternalOutput").ap()

    KC = D // 128  # 32 contraction chunks
    SLAB = 4
    NSLAB = KC // SLAB

    with tile.TileContext(nc) as tc, ExitStack() as s0:
        kvp = s0.enter_context(tc.tile_pool(name="kv", bufs=1))
        # persistent SBUF-resident intermediates
        Qh = [kvp.tile([128, NT], bf16, tag=f"Q{h}", name=f"Q{h}") for h in range(HPC)]
        KTb = [kvp.tile([128, T], bf16, tag=f"KT{i}", name=f"KT{i}") for i in range(B)]
        Vb = [kvp.tile([128, T], bf16, tag=f"V{i}", name=f"V{i}") for i in range(B)]
        Ch = [kvp.tile([128, NT], bf16, tag=f"C{h}", name=f"C{h}") for h in range(HPC)]
        tabA = kvp.tile([128, T], bf16, tag="tabA")
        tabB = kvp.tile([128, T], bf16, tag="tabB")
        tri = kvp.tile([128, 128], bf16, tag="tri")

        wp = s0.enter_context(tc.tile_pool(name="wts", bufs=1))
        wq_sb = wp.tile([128, KC * FPC], bf16, tag="wq")
        wk_sb = wp.tile([128, KC * DH], bf16, tag="wk")
        wv_sb = wp.tile([128, KC * DH], bf16, tag="wv")
        xsp = s0.enter_context(tc.tile_pool(name="xs", bufs=3))
        evp = s0.enter_context(tc.tile_pool(name="ev", bufs=1))
        # unified PSUM ring: tag "a" = 6 banks (projection accumulators /
        # score ring / output-projection / denominator), tag "ctx" = 2 banks
        # (V transposes in phase A, ctx accumulators in phase B)
        psU = s0.enter_context(tc.tile_pool(name="psU", bufs=1, space="PSUM"))
        # phase B pools (allocated up front; no barrier between phases)
        obp = s0.enter_context(tc.tile_pool(name="obp", bufs=6))
        sxp = s0.enter_context(tc.tile_pool(name="sxp", bufs=8))
        smp = s0.enter_context(tc.tile_pool(name="smp", bufs=2))
        accp = s0.enter_context(tc.tile_pool(name="accp", bufs=2))

        def load_w_slab(s, engines=None):
            # one consolidated DMA per weight tensor per 4-chunk slab
            e = engines or (nc.sync, nc.sync, nc.sync)
            e[0].dma_start(
                wq_sb[:, s * SLAB * FPC:(s + 1) * SLAB * FPC]
                .rearrange("p (k c) -> p k c", k=SLAB),
                wq[s * 512:(s + 1) * 512, :]
                .rearrange("(k p) c -> p k c", k=SLAB))
            e[1].dma_start(
                wk_sb[:, s * SLAB * DH:(s + 1) * SLAB * DH]
                .rearrange("p (k c) -> p k c", k=SLAB),
                wk[s * 512:(s + 1) * 512, :]
                .rearrange("(k p) c -> p k c", k=SLAB))
            e[2].dma_start(
                wv_sb[:, s * SLAB * DH:(s + 1) * SLAB * DH]
                .rearrange("p (k c) -> p k c", k=SLAB),
                wv[s * 512:(s + 1) * 512, :]
                .rearrange("(k p) c -> p k c", k=SLAB))

        def load_xsl(n, s):
            xsl = xsp.tile([128, SLAB * 512], bf16, tag="xs", name="xsl")
            nc.sync.dma_start(
                xsl[:].rearrange("p (k c) -> p k c", k=SLAB),
                xT[s * 512:(s + 1) * 512, n * 512:(n + 1) * 512]
                .rearrange("(k p) c -> p k c", k=SLAB))
            return xsl

        # critical-path first: interleave slab-0 weight and x chunks at
        # 128-row granularity, issued from four engine queues in parallel so
        # the sync-queue descriptor rate doesn't gate the pipeline
        xsl0 = xsp.tile([128, SLAB * 512], bf16, tag="xs", name="xsl")
        for k in range(SLAB):
            nc.sync.dma_start(wq_sb[:, k * FPC:(k + 1) * FPC],
                              wq[k * 128:(k + 1) * 128, :])
            nc.scalar.dma_start(xsl0[:, k * 512:(k + 1) * 512],
                                xT[k * 128:(k + 1) * 128, 0:512])
            nc.gpsimd.dma_start(wk_sb[:, k * DH:(k + 1) * DH],
                                wk[k * 128:(k + 1) * 128, :])
            nc.gpsimd.dma_start(wv_sb[:, k * DH:(k + 1) * DH],
                                wv[k * 128:(k + 1) * 128, :])
        load_w_slab(1, engines=(nc.scalar, nc.gpsimd, nc.sync))

        # setup that doesn't gate the first matmuls
        ident_f = kvp.tile([128, 128], f32, tag="ident_f")
        make_identity(nc, ident_f[:])
        ones_f = kvp.tile([128, 128], f32, tag="ones_f")
        nc.vector.memset(ones_f[:], 1.0)
        ones128 = kvp.tile([128, 128], bf16, tag="ones128")
        nc.vector.tensor_copy(ones128[:], ones_f[:])

        # ---------------- Phase A: projections + RoPE -----------------
        def stationary(m, k):
            if m < HPC:
                return wq_sb[:, k * FPC + m * 128: k * FPC + (m + 1) * 128]
            if m == HPC:
                return wk_sb[:, k * DH:(k + 1) * DH]
            return wv_sb[:, k * DH:(k + 1) * DH]

        pending_vt = None

        def flush_vt():
            nonlocal pending_vt
            if pending_vt is None:
                return
            vt_p, n_p = pending_vt
            b_p = n_p // 4
            for i in range(4):
                # f32 transpose into a "ctx" PSUM slot (unused in phase A)
                ptr = psU.tile([128, 512], f32, tag="ctx", bufs=2, name="ptr")
                nc.tensor.transpose(ptr[:, 0:128], vt_p[:, i * 128:(i + 1) * 128],
                                    ident_f[:])
                c_local = 4 * (n_p % 4) + i
                nc.scalar.copy(Vb[b_p][:, c_local * 128:c_local * 128 + 128],
                               ptr[:, 0:128])
            pending_vt = None

        for n in range(NT // 512):
            b, tloc = n // 4, 512 * (n % 4)
            ps = [psU.tile([128, 512], f32, tag="a", bufs=6, name=f"ps{m}")
                  for m in range(6)]
            for s in range(NSLAB):
                if n == 0:
                    xsl = xsl0 if s == 0 else xsl_next
                    if s + 1 < NSLAB:
                        xsl_next = load_xsl(0, s + 1)
                    if s + 2 < NSLAB:
                        load_w_slab(s + 2)
                else:
                    xsl = load_xsl(n, s)
                for m in range(6):
                    for j in range(SLAB):
                        k = s * SLAB + j
                        nc.tensor.matmul(ps[m][:], stationary(m, k),
                                         xsl[:, j * 512:(j + 1) * 512],
                                         start=(k == 0), stop=(k == KC - 1))
                if s == 0:
                    flush_vt()   # prev n-tile's V transposes, PE already warm here
            if n == 0:
                nc.sync.dma_start(tabA[:], ropeA)
                nc.sync.dma_start(tabB[:], ropeB)
            if n == 1:
                nc.sync.dma_start(tri[:], trim)
            # evict: split across ACT and DVE so the PSUM banks drain in
            # parallel
            qes = []
            for m in range(5):
                qe = evp.tile([128, 512], bf16, tag="qe", bufs=6, name=f"qe{m}")
                if m < 3:
                    nc.scalar.copy(qe[:], ps[m][:])
                else:
                    nc.vector.tensor_copy(qe[:], ps[m][:])
                qes.append(qe)
            vt = evp.tile([128, 512], f32, tag="vt", bufs=2, name="vt")
            nc.scalar.copy(vt[:], ps[5][:])
            pending_vt = (vt, n)
            # rope chains on DVE (bf16, 2x rate)
            tA = tabA[:, tloc:tloc + 512]
            tB = tabB[:, tloc:tloc + 512]
            for m in range(5):
                qe = qes[m]
                sw = evp.tile([128, 512], bf16, tag="sw", bufs=1, name="sw")
                nc.vector.tensor_copy(sw[0:64, :], qe[64:128, :])
                nc.vector.tensor_copy(sw[64:128, :], qe[0:64, :])
                mm = evp.tile([128, 512], bf16, tag="mm", bufs=1, name="mm")
                nc.vector.tensor_mul(mm[:], sw[:], tB)
                tt = evp.tile([128, 512], bf16, tag="tt", bufs=1, name="tt")
                nc.vector.tensor_mul(tt[:], qe[:], tA)
                if m < HPC:
                    nc.vector.tensor_add(Qh[m][:, n * 512:(n + 1) * 512], tt[:], mm[:])
                else:
                    nc.vector.tensor_add(KTb[b][:, tloc:tloc + 512], tt[:], mm[:])
        flush_vt()

        # Wo prefetch into Wq's SBUF ring slot (same shape, bufs=1): the DMA
        # naturally waits for the last Q-projection matmul, no barrier needed.
        wo_sb = wp.tile([128, KC * FPC], bf16, tag="wq", name="wo_sb")
        wo_e = (nc.sync, nc.scalar, nc.gpsimd, nc.sync)
        for h in range(HPC):
            wo_e[h].dma_start(wo_sb[:, h * D:(h + 1) * D], wo[h * 128:(h + 1) * 128, :])

        # ---------------- Phase B: attention -----------------
        pending_norm = None
        cols_since_flush = 0

        def flush_norm():
            # stage 2: multiply ps_ctx by the reduced+broadcast reciprocal
            nonlocal pending_norm
            if pending_norm is None:
                return
            ps_ctx_p, rec_p, h_p, b_p, qt_p = pending_norm
            nc.vector.tensor_mul(
                Ch[h_p][:, b_p * T + qt_p * 512: b_p * T + (qt_p + 1) * 512],
                ps_ctx_p[:], rec_p[:])
            pending_norm = None
            # h==3 with qt-major order: once the last head of a (b,qt) is
            # flushed, those token blocks are complete across all heads and
            # their output-projection groups may be emitted
            nonlocal c_limit
            if h_p == 3:
                c_limit = 8 * (16 * b_p + 4 * (qt_p + 1))

        # Flat global iteration stream, qt-major so output-projection
        # groups unlock as soon as all 4 heads finish a (b,qt).
        # item = (b, h, qt, kt, q-offset, width, idx-in-qt, n-in-qt)
        items = []
        for b in range(B):
            for qt in range(4):
                for h in range(HPC):
                    plan = [(kt, 0, 512) for kt in range(4 * qt)]
                    plan += [(4 * qt + r, 128 * r, 512 - 128 * r) for r in range(4)]
                    for i, (kt, off, w) in enumerate(plan):
                        items.append((b, h, qt, kt, off, w, i, len(plan)))

        def issue_st(g):
            b, h, qt, kt, off, w, i, npl = items[g]
            ps_st = psU.tile([128, 512], f32, tag="a", bufs=6, name="ps_st")
            nc.tensor.matmul(ps_st[:, 0:w],
                             KTb[b][:, kt * 128:(kt + 1) * 128],
                             Qh[h][:, b * T + qt * 512 + off: b * T + qt * 512 + off + w],
                             start=True, stop=True)
            if kt - 4 * qt >= 0:
                # diagonal block: additive causal mask (-2e4 on the
                # strictly-lower triangle) folded into PSUM on DVE; the
                # depth-3 prefetch hides the extra hop before the exp
                nc.vector.tensor_add(ps_st[:, 0:128], ps_st[:, 0:128], tri[:])
            se = sxp.tile([128, 512], bf16, tag="se", name="se")
            nc.scalar.activation(se[:, 0:w], ps_st[:, 0:w],
                                 mybir.ActivationFunctionType.Exp,
                                 scale=EXP_SCALE)
            return se

        # Phase C output-projection groups, interleaved into the attention
        # stream as PE filler once their tokens' ctx is complete
        c_groups = [(m, n_) for m in range(NT // 128) for n_ in range(D // 512)]
        c_cursor = 0
        c_limit = 0
        C_START_MIN = 48  # don't emit before the Wo DMA has surely landed

        def emit_c_group(drain=False):
            nonlocal c_cursor
            if c_cursor >= len(c_groups):
                return
            m, n_ = c_groups[c_cursor]
            gi = c_cursor
            c_cursor += 1
            pso = psU.tile([128, 512], f32, tag="a", bufs=6, name="pso")
            for hh in range(HPC):
                nc.tensor.matmul(pso[:], Ch[hh][:, m * 128:(m + 1) * 128],
                                 wo_sb[:, hh * D + n_ * 512: hh * D + (n_ + 1) * 512],
                                 start=(hh == 0), stop=(hh == HPC - 1))
            ob = obp.tile([128, 512], bf16, tag="ob", name="ob")
            # alternate eviction engines so neither the exp-laden ACT queue
            # nor the flush-laden DVE queue becomes the bottleneck
            if gi % 2:
                nc.vector.tensor_copy(ob[:], pso[:])
            else:
                nc.scalar.copy(ob[:], pso[:])
            if drain:
                # split across two queues: the end burst is DMA-drain bound
                nc.sync.dma_start(outp[gi, 0:64], ob[0:64, :])
                nc.scalar.dma_start(outp[gi, 64:128], ob[64:128, :])
            else:
                nc.sync.dma_start(outp[gi], ob[:])

        # one-time 4-deep seed fills the cold exp chain at attention
        # start; the issue cursor then settles to sustainable depth 3
        se_q = [issue_st(0), issue_st(1), issue_st(2), issue_st(3)]
        next_issue = 4
        ps_ctx = acc_e = acc_o = None
        den_queue = []  # (ready_g, accb_e, accb_o, ps_ctx, h, b, qt)

        def pop_den():
            # deferred ~2 items so the casts are done before the PE hits
            # the reduction matmuls; all-ones stationary reduces across
            # partitions AND broadcasts the sums to all 128 output partitions
            nonlocal pending_norm, cols_since_flush
            _, ab_e, ab_o, ps_ctx_p, h_p, b_p, qt_p = den_queue.pop(0)
            den_ps = psU.tile([128, 512], f32, tag="a", bufs=6, name="den")
            nc.tensor.matmul(den_ps[:], ones128[:], ab_e[:],
                             start=True, stop=False, skip_group_check=True)
            nc.tensor.matmul(den_ps[:], ones128[:], ab_o[:],
                             start=False, stop=True, skip_group_check=True)
            flush_norm()  # finish the previous (b,qt,h) if still pending
            rec = smp.tile([128, 512], f32, tag="rec", name="rec")
            nc.vector.reciprocal_approx_fast(out=rec[:], in_=den_ps[:])
            pending_norm = (ps_ctx_p, rec, h_p, b_p, qt_p)
            cols_since_flush = 0

        for g, (b, h, qt, kt, off, w, i, npl) in enumerate(items):
            se_cur = se_q.pop(0)
            if next_issue < len(items) and next_issue <= g + 3:
                se_q.append(issue_st(next_issue))
                next_issue += 1
            if i == 0:
                flush_norm()  # safety: free the other ctx bank before realloc
                ps_ctx = psU.tile([128, 512], f32, tag="ctx", bufs=2, name="ps_ctx")
            last = (i == npl - 1)
            nc.tensor.matmul(ps_ctx[:, off:off + w],
                             Vb[b][:, kt * 128:(kt + 1) * 128],
                             se_cur[:, 0:w],
                             start=(i == 0), stop=last,
                             skip_group_check=True)
            # softmax denominator accumulated off the PE, split into two
            # independent chains: even items on DVE, odd items on gpsimd
            # (both SBUF-only, so the Pool engine is legal here)
            if i == 0:
                acc_e = accp.tile([128, 512], f32, tag="acce", name="acce")
                nc.vector.tensor_copy(acc_e[:], se_cur[:])
            elif i == 1:
                acc_o = accp.tile([128, 512], f32, tag="acco", name="acco")
                nc.gpsimd.tensor_copy(acc_o[:, off:off + w], se_cur[:, 0:w])
                if off:
                    nc.gpsimd.memset(acc_o[:, 0:off], 0.0)
            elif i % 2 == 0:
                nc.vector.tensor_add(acc_e[:, off:off + w], acc_e[:, off:off + w],
                                     se_cur[:, 0:w])
            else:
                nc.gpsimd.tensor_add(acc_o[:, off:off + w], acc_o[:, off:off + w],
                                     se_cur[:, 0:w])
            if den_queue and den_queue[0][0] <= g:
                pop_den()
            cols_since_flush += 2 * w
            if pending_norm is not None and cols_since_flush >= 1500:
                flush_norm()
            if c_cursor < c_limit and g >= C_START_MIN:
                emit_c_group()
                if c_limit - c_cursor >= 8 and c_cursor < c_limit:
                    emit_c_group()
            if last:
                ab_e = accp.tile([128, 512], bf16, tag="abe", name="abe")
                nc.vector.tensor_copy(ab_e[:], acc_e[:])
                ab_o = accp.tile([128, 512], bf16, tag="abo", name="abo")
                nc.gpsimd.tensor_copy(ab_o[:], acc_o[:])
                den_queue.append((g + 2, ab_e, ab_o, ps_ctx, h, b, qt))
        while den_queue:
            pop_den()
        flush_norm()
        # remaining output-projection groups
        while c_cursor < len(c_groups):
            emit_c_group(drain=True)

    nc.compile()
    return nc


def _get_nc():
    if "nc" not in _NC_CACHE:
        _NC_CACHE["nc"] = _build_program()
    return _NC_CACHE["nc"]


def _rope_tables():
    j = np.arange(0, DH, 2, dtype=np.float32) / np.float32(DH)
    inv_freq = (np.float32(1.0) / (np.float32(ROPE_BASE) ** j)).astype(np.float32)
    t = np.arange(T, dtype=np.float32)
    freqs = np.outer(t, inv_freq).astype(np.float32)   # (T, 64)
    c = np.cos(freqs).astype(np.float32).T             # (64, T)
    s = np.sin(freqs).astype(np.float32).T
    A = np.vstack([c, c]).astype(np.float32)           # (128, T)
    Bt = np.vstack([-s, s]).astype(np.float32)
    return np.ascontiguousarray(A).astype(BF), np.ascontiguousarray(Bt).astype(BF)


def _tri_mask():
    # additive causal mask for a diagonal 128x128 block: key p visible to
    # query j iff p <= j; exp((s - 2e4) * scale) == 0 otherwise
    p = np.arange(128)[:, None]
    j = np.arange(128)[None, :]
    return np.where(p <= j, 0.0, -20000.0).astype(np.float32).astype(BF)


def _make_in_maps(x, Wq, Wk, Wv, Wo):
    xT = np.ascontiguousarray(x.reshape(NT, D).T).astype(BF)
    A, Bt = _rope_tables()
    tri = _tri_mask()

    in_maps = []
    for g in range(8):
        in_maps.append({
            "xT": xT,
            "wq": np.ascontiguousarray(Wq[:, g * FPC:(g + 1) * FPC]).astype(BF),
            "wk": np.ascontiguousarray(Wk[:, g * DH:(g + 1) * DH]).astype(BF),
            "wv": np.ascontiguousarray(Wv[:, g * DH:(g + 1) * DH]).astype(BF),
            "wo": np.ascontiguousarray(Wo[g * FPC:(g + 1) * FPC, :]).astype(BF),
            "ropeA": A,
            "ropeB": Bt,
            "trim": tri,
        })
    return in_maps


def kernel(x, Wq, Wk, Wv, Wo):
    x = np.asarray(x, dtype=np.float32)
    Wq = np.asarray(Wq, dtype=np.float32)
    Wk = np.asarray(Wk, dtype=np.float32)
    Wv = np.asarray(Wv, dtype=np.float32)
    Wo = np.asarray(Wo, dtype=np.float32)

    nc = _get_nc()
    in_maps = _make_in_maps(x, Wq, Wk, Wv, Wo)

    res = run_bass_kernel_spmd(nc, in_maps, list(range(8)))
    acc = res.results[0]["outp"].astype(np.float32)
    for g in range(1, 8):
        acc = acc + res.results[g]["outp"].astype(np.float32)
    # outp blocks: gi = m*8 + n_ -> out[m*128+p, n_*512+c]
    out = acc.reshape(NT // 128, D // 512, 128, 512).transpose(0, 2, 1, 3)
    return np.ascontiguousarray(out.reshape(B, T, D), dtype=np.float32)


# revision 13
# speedup vs baseline: 1.0040x; 1.0040x over previous
"""GroupedQueryAttention TRN2 Bass kernel, sharded over 8 NeuronCores.

Problem (hardcoded): B=2, T=2048, D=4096, 32 Q heads x 128, 8 KV groups x 128,
RoPE (base 5e5), causal, out = ctx @ Wo.

Sharding: core g owns Q heads 4g..4g+3 (Wq columns 512g:512g+512), KV group g
(Wk/Wv columns 128g:128g+128), and Wo rows 512g:512g+512 (row-parallel).
Each core computes a full-shape partial output; host sums the 8 partials.

v5 (over v4): single pool scope with a unified PSUM ring — no pool-close
barrier between projection and attention phases, so the attention seeds
overlap the last projection tile's RoPE tail; softmax denominator reduction
via an all-ones [128,128] stationary matmul that reduces AND broadcasts in
one op (drops the gpsimd broadcast from the critical path); output-projection
evictions moved off the exp-laden ACT queue (DVE/gpsimd rotate); Wo reuses
Wq's SBUF ring slot; startup DMAs issued from four engine queues in
parallel; drain-phase output DMAs split across two queues.
"""
import sys
import numpy as np

for _p in ("/opt/trn_rl_repo", "/root/.axon_site", "/root/.axon_site/_ro/trn_rl_repo"):
    if _p not in sys.path:
        sys.path.append(_p)

from contextlib import ExitStack

import ml_dtypes

import concourse.bass as bass
import concourse.tile as tile
from concourse import bacc, mybir
from concourse.bass_utils import run_bass_kernel_spmd
from concourse.masks import make_identity

B, T, D = 2, 2048, 4096
NH, NKV, DH = 32, 8, 128
HPC = NH // 8          # 4 q heads per core
FPC = HPC * DH         # 512 q features per core
ROPE_BASE = 500000.0
NT = B * T             # 4096 tokens
f32 = mybir.dt.float32
bf16 = mybir.dt.bfloat16
EXP_SCALE = 1.0 / float(np.sqrt(DH))
BF = ml_dtypes.bfloat16

_NC_CACHE = {}


def _build_program():
    nc = bacc.Bacc("TRN2", target_bir_lowering=False, debug=False)

    xT = nc.dram_tensor("xT", [D, NT], bf16, kind="ExternalInput").ap()
    wq = nc.dram_tensor("wq", [D, FPC], bf16, kind="ExternalInput").ap()
    wk = nc.dram_tensor("wk", [D, DH], bf16, kind="ExternalInput").ap()
    wv = nc.dram_tensor("wv", [D, DH], bf16, kind="ExternalInput").ap()
    wo = nc.dram_tensor("wo", [FPC, D], bf16, kind="ExternalInput").ap()
    ropeA = nc.dram_tensor("ropeA", [128, T], bf16, kind="ExternalInput").ap()
    ropeB = nc.dram_tensor("ropeB", [128, T], bf16, kind="ExternalInput").ap()
    trim = nc.dram_tensor("trim", [128, 128], bf16, kind="ExternalInput").ap()
    # output stored as contiguous [128,512] blocks: gi = m*8 + n_
    outp = nc.dram_tensor("outp", [(NT // 128) * (D // 512), 128, 512], bf16,
                          kind="ExternalOutput").ap()

    KC = D // 128  # 32 contraction chunks
    SLAB = 4
    NSLAB = KC // SLAB

    with tile.TileContext(nc) as tc, ExitStack() as s0:
        kvp = s0.enter_context(tc.tile_pool(name="kv", bufs=1))
        # persistent SBUF-resident intermediates
        Qh = [[kvp.tile([128, T], bf16, tag=f"Q{h}b{i}", name=f"Q{h}b{i}")
               for i in range(B)] for h in range(HPC)]
        KTb = [kvp.tile([128, T], bf16, tag=f"KT{i}", name=f"KT{i}") for i in range(B)]
        Vb = [kvp.tile([128, T], bf16, tag=f"V{i}", name=f"V{i}") for i in range(B)]
        Ch = [kvp.tile([128, NT], bf16, tag=f"C{h}", name=f"C{h}") for h in range(HPC)]
        tabA = kvp.tile([128, T], bf16, tag="tabA")
        tabB = kvp.tile([128, T], bf16, tag="tabB")
        tri = kvp.tile([128, 128], bf16, tag="tri")

        wp = s0.enter_context(tc.tile_pool(name="wts", bufs=1))
        wq_sb = wp.tile([128, KC * FPC], bf16, tag="wq")
        wk_sb = wp.tile([128, KC * DH], bf16, tag="wk")
        wv_sb = wp.tile([128, KC * DH], bf16, tag="wv")
        xsp = s0.enter_context(tc.tile_pool(name="xs", bufs=3))
        evp = s0.enter_context(tc.tile_pool(name="ev", bufs=1))
        # unified PSUM ring: tag "a" = 6 banks (projection accumulators /
        # score ring / output-projection / denominator), tag "ctx" = 2 banks
        # (V transposes in phase A, ctx accumulators in phase B)
        psU = s0.enter_context(tc.tile_pool(name="psU", bufs=1, space="PSUM"))
        # phase B pools (allocated up front; no barrier between phases)
        obp = s0.enter_context(tc.tile_pool(name="obp", bufs=6))
        sxp = s0.enter_context(tc.tile_pool(name="sxp", bufs=8))
        smp = s0.enter_context(tc.tile_pool(name="smp", bufs=2))
        accp = s0.enter_context(tc.tile_pool(name="accp", bufs=2))

        def load_w_slab(s, engines=None):
            # one consolidated DMA per weight tensor per 4-chunk slab
            e = engines or (nc.sync, nc.sync, nc.sync)
            e[0].dma_start(
                wq_sb[:, s * SLAB * FPC:(s + 1) * SLAB * FPC]
                .rearrange("p (k c) -> p k c", k=SLAB),
                wq[s * 512:(s + 1) * 512, :]
                .rearrange("(k p) c -> p k c", k=SLAB))
            e[1].dma_start(
                wk_sb[:, s * SLAB * DH:(s + 1) * SLAB * DH]
                .rearrange("p (k c) -> p k c", k=SLAB),
                wk[s * 512:(s + 1) * 512, :]
                .rearrange("(k p) c -> p k c", k=SLAB))
            e[2].dma_start(
                wv_sb[:, s * SLAB * DH:(s + 1) * SLAB * DH]
                .rearrange("p (k c) -> p k c", k=SLAB),
                wv[s * 512:(s + 1) * 512, :]
                .rearrange("(k p) c -> p k c", k=SLAB))

        def load_xsl(n, s):
            xsl = xsp.tile([128, SLAB * 512], bf16, tag="xs", name="xsl")
            nc.sync.dma_start(
                xsl[:].rearrange("p (k c) -> p k c", k=SLAB),
                xT[s * 512:(s + 1) * 512, n * 512:(n + 1) * 512]
                .rearrange("(k p) c -> p k c", k=SLAB))
            return xsl

        # critical-path first: interleave slab-0 weight and x chunks at
        # 128-row granularity, issued from four engine queues in parallel so
        # the sync-queue descriptor rate doesn't gate the pipeline
        xsl0 = xsp.tile([128, SLAB * 512], bf16, tag="xs", name="xsl")
        for k in range(SLAB):
            nc.sync.dma_start(wq_sb[:, k * FPC:(k + 1) * FPC],
                              wq[k * 128:(k + 1) * 128, :])
            nc.scalar.dma_start(xsl0[:, k * 512:(k + 1) * 512],
                                xT[k * 128:(k + 1) * 128, 0:512])
            nc.gpsimd.dma_start(wk_sb[:, k * DH:(k + 1) * DH],
                                wk[k * 128:(k + 1) * 128, :])
            nc.gpsimd.dma_start(wv_sb[:, k * DH:(k + 1) * DH],
                                wv[k * 128:(k + 1) * 128, :])
        load_w_slab(1, engines=(nc.scalar, nc.gpsimd, nc.sync))

        # setup that doesn't gate the first matmuls
        ident_f = kvp.tile([128, 128], f32, tag="ident_f")
        make_identity(nc, ident_f[:])
        ones_f = kvp.tile([128, 128], f32, tag="ones_f")
        nc.vector.memset(ones_f[:], 1.0)
        ones128 = kvp.tile([128, 128], bf16, tag="ones128")
        nc.vector.tensor_copy(ones128[:], ones_f[:])

        # ---------------- Phase A: projections + RoPE -----------------
        def stationary(m, k):
            if m < HPC:
                return wq_sb[:, k * FPC + m * 128: k * FPC + (m + 1) * 128]
            if m == HPC:
                return wk_sb[:, k * DH:(k + 1) * DH]
            return wv_sb[:, k * DH:(k + 1) * DH]

        pending_vt = None

        def flush_vt():
            nonlocal pending_vt
            if pending_vt is None:
                return
            vt_p, n_p = pending_vt
            b_p = n_p // 4
            for i in range(4):
                # f32 transpose into a "ctx" PSUM slot (unused in phase A)
                ptr = psU.tile([128, 512], f32, tag="ctx", bufs=2, name="ptr")
                nc.tensor.transpose(ptr[:, 0:128], vt_p[:, i * 128:(i + 1) * 128],
                                    ident_f[:])
                c_local = 4 * (n_p % 4) + i
                nc.scalar.copy(Vb[b_p][:, c_local * 128:c_local * 128 + 128],
                               ptr[:, 0:128])
            pending_vt = None

        for n in range(NT // 512):
            b, tloc = n // 4, 512 * (n % 4)
            ps = [psU.tile([128, 512], f32, tag="a", bufs=6, name=f"ps{m}")
                  for m in range(6)]
            for s in range(NSLAB):
                if n == 0:
                    xsl = xsl0 if s == 0 else xsl_next
                    if s + 1 < NSLAB:
                        xsl_next = load_xsl(0, s + 1)
                    if s + 2 < NSLAB:
                        load_w_slab(s + 2)
                else:
                    xsl = load_xsl(n, s)
                for m in range(6):
                    for j in range(SLAB):
                        k = s * SLAB + j
                        nc.tensor.matmul(ps[m][:], stationary(m, k),
                                         xsl[:, j * 512:(j + 1) * 512],
                                         start=(k == 0), stop=(k == KC - 1))
                if s == 0:
                    flush_vt()   # prev n-tile's V transposes, PE already warm here
            if n == 0:
                nc.sync.dma_start(tabA[:], ropeA)
                nc.sync.dma_start(tabB[:], ropeB)
            if n == 1:
                nc.sync.dma_start(tri[:], trim)
            # evict: split across ACT and DVE so the PSUM banks drain in
            # parallel
            qes = []
            for m in range(5):
                qe = evp.tile([128, 512], bf16, tag="qe", bufs=6, name=f"qe{m}")
                if m < 3:
                    nc.scalar.copy(qe[:], ps[m][:])
                else:
                    nc.vector.tensor_copy(qe[:], ps[m][:])
                qes.append(qe)
            vt = evp.tile([128, 512], f32, tag="vt", bufs=2, name="vt")
            nc.scalar.copy(vt[:], ps[5][:])
            pending_vt = (vt, n)
            # rope chains on DVE (bf16, 2x rate)
            tA = tabA[:, tloc:tloc + 512]
            tB = tabB[:, tloc:tloc + 512]
            for m in range(5):
                qe = qes[m]
                sw = evp.tile([128, 512], bf16, tag="sw", bufs=1, name="sw")
                nc.vector.tensor_copy(sw[0:64, :], qe[64:128, :])
                nc.vector.tensor_copy(sw[64:128, :], qe[0:64, :])
                mm = evp.tile([128, 512], bf16, tag="mm", bufs=1, name="mm")
                nc.vector.tensor_mul(mm[:], sw[:], tB)
                tt = evp.tile([128, 512], bf16, tag="tt", bufs=1, name="tt")
                nc.vector.tensor_mul(tt[:], qe[:], tA)
                if m < HPC:
                    nc.vector.tensor_add(Qh[m][b][:, tloc:tloc + 512], tt[:], mm[:])
                else:
                    nc.vector.tensor_add(KTb[b][:, tloc:tloc + 512], tt[:], mm[:])
        flush_vt()

        # Wo prefetch into Wq's SBUF ring slot (same shape, bufs=1): the DMA
        # naturally waits for the last Q-projection matmul, no barrier needed.
        wo_sb = wp.tile([128, KC * FPC], bf16, tag="wq", name="wo_sb")
        wo_e = (nc.sync, nc.scalar, nc.gpsimd, nc.sync)
        for h in range(HPC):
            wo_e[h].dma_start(wo_sb[:, h * D:(h + 1) * D], wo[h * 128:(h + 1) * 128, :])

        # ---------------- Phase B: attention -----------------

        # Flat global iteration stream, qt-major so output-projection
        # groups unlock as soon as all 4 heads finish a (b,qt).
        # item = (b, h, qt, kt, q-offset, width, idx-in-qt, n-in-qt)
        items = []
        for b in range(B):
            for qt in range(4):
                for h in range(HPC):
                    plan = [(kt, 0, 512) for kt in range(4 * qt)]
                    plan += [(4 * qt + r, 128 * r, 512 - 128 * r) for r in range(4)]
                    for i, (kt, off, w) in enumerate(plan):
                        items.append((b, h, qt, kt, off, w, i, len(plan)))

        def issue_st(g):
            b, h, qt, kt, off, w, i, npl = items[g]
            ps_st = psU.tile([128, 512], f32, tag="a", bufs=6, name="ps_st")
            nc.tensor.matmul(ps_st[:, 0:w],
                             KTb[b][:, kt * 128:(kt + 1) * 128],
                             Qh[h][b][:, qt * 512 + off: qt * 512 + off + w],
                             start=True, stop=True)
            if kt - 4 * qt >= 0:
                # diagonal block: additive causal mask (-2e4 on the
                # strictly-lower triangle) folded into PSUM on DVE; the
                # depth-3 prefetch hides the extra hop before the exp
                nc.vector.tensor_add(ps_st[:, 0:128], ps_st[:, 0:128], tri[:])
            se = sxp.tile([128, 512], bf16, tag="se", name="se")
            nc.scalar.activation(se[:, 0:w], ps_st[:, 0:w],
                                 mybir.ActivationFunctionType.Exp,
                                 scale=EXP_SCALE)
            return se

        # Phase C output-projection groups, interleaved into the attention
        # stream as PE filler once their tokens' ctx is complete
        c_groups = [(m, n_) for m in range(NT // 128) for n_ in range(D // 512)]
        c_cursor = 0
        c_limit = 0
        C_START_MIN = 48  # don't emit before the Wo DMA has surely landed

        def emit_c_group(drain=False):
            nonlocal c_cursor
            if c_cursor >= len(c_groups):
                return
            m, n_ = c_groups[c_cursor]
            gi = c_cursor
            c_cursor += 1
            pso = psU.tile([128, 512], f32, tag="a", bufs=6, name="pso")
            for hh in range(HPC):
                nc.tensor.matmul(pso[:], Ch[hh][:, m * 128:(m + 1) * 128],
                                 wo_sb[:, hh * D + n_ * 512: hh * D + (n_ + 1) * 512],
                                 start=(hh == 0), stop=(hh == HPC - 1))
            ob = obp.tile([128, 512], bf16, tag="ob", name="ob")
            if drain:
                # endgame: ACT/exp is done; keep evictions on DVE only and
                # spread DMA descriptor issue across two idle queues
                nc.vector.tensor_copy(ob[:], pso[:])
                (nc.sync if gi % 2 else nc.gpsimd).dma_start(outp[gi], ob[:])
            else:
                # alternate eviction engines so neither the exp-laden ACT
                # queue nor the flush-laden DVE queue becomes the bottleneck
                if gi % 2:
                    nc.vector.tensor_copy(ob[:], pso[:])
                else:
                    nc.scalar.copy(ob[:], pso[:])
                nc.sync.dma_start(outp[gi], ob[:])

        # one-time 4-deep seed fills the cold exp chain at attention
        # start; the issue cursor then settles to sustainable depth 3
        se_q = [issue_st(0), issue_st(1), issue_st(2), issue_st(3)]
        next_issue = 4
        ps_ctx = acc_e = acc_o = None
        den_queue = []  # (ready_g, accb_e, accb_o, ps_ctx, h, b, qt)

        def pop_den():
            # deferred 3 items so the DVE/gpsimd casts are done before the
            # PE hits the reduction matmuls; all-ones stationary reduces
            # across partitions AND broadcasts to all 128 output partitions,
            # so the normalization multiply can follow immediately
            nonlocal c_limit
            _, ab_e, ab_o, ps_ctx_p, h_p, b_p, qt_p = den_queue.pop(0)
            den_ps = psU.tile([128, 512], f32, tag="a", bufs=6, name="den")
            nc.tensor.matmul(den_ps[:], ones128[:], ab_e[:],
                             start=True, stop=False, skip_group_check=True)
            nc.tensor.matmul(den_ps[:], ones128[:], ab_o[:],
                             start=False, stop=True, skip_group_check=True)
            rec = smp.tile([128, 512], f32, tag="rec", name="rec")
            nc.vector.reciprocal_approx_fast(out=rec[:], in_=den_ps[:])
            nc.vector.tensor_mul(
                Ch[h_p][:, b_p * T + qt_p * 512: b_p * T + (qt_p + 1) * 512],
                ps_ctx_p[:], rec[:])
            # h==3 with qt-major order: those token blocks are now complete
            # across all heads; their output-projection groups may be emitted
            if h_p == 3:
                c_limit = 8 * (16 * b_p + 4 * (qt_p + 1))

        for g, (b, h, qt, kt, off, w, i, npl) in enumerate(items):
            se_cur = se_q.pop(0)
            if next_issue < len(items) and next_issue <= g + 3:
                se_q.append(issue_st(next_issue))
                next_issue += 1
            if i == 0:
                ps_ctx = psU.tile([128, 512], f32, tag="ctx", bufs=2, name="ps_ctx")
            last = (i == npl - 1)
            nc.tensor.matmul(ps_ctx[:, off:off + w],
                             Vb[b][:, kt * 128:(kt + 1) * 128],
                             se_cur[:, 0:w],
                             start=(i == 0), stop=last,
                             skip_group_check=True)
            # softmax denominator accumulated off the PE, split into two
            # independent chains: even items on DVE, odd items on gpsimd
            # (both SBUF-only, so the Pool engine is legal here)
            if i == 0:
                acc_e = accp.tile([128, 512], f32, tag="acce", name="acce")
                nc.vector.tensor_copy(acc_e[:], se_cur[:])
            elif i == 1:
                acc_o = accp.tile([128, 512], f32, tag="acco", name="acco")
                nc.gpsimd.tensor_copy(acc_o[:, off:off + w], se_cur[:, 0:w])
                if off:
                    nc.gpsimd.memset(acc_o[:, 0:off], 0.0)
            elif i % 2 == 0:
                nc.vector.tensor_add(acc_e[:, off:off + w], acc_e[:, off:off + w],
                                     se_cur[:, 0:w])
            else:
                nc.gpsimd.tensor_add(acc_o[:, off:off + w], acc_o[:, off:off + w],
                                     se_cur[:, 0:w])
            if den_queue and den_queue[0][0] <= g:
                pop_den()
            if c_cursor < c_limit and g >= C_START_MIN:
                emit_c_group()
                if c_limit - c_cursor >= 8 and c_cursor < c_limit:
                    emit_c_group()
            if last:
                ab_e = accp.tile([128, 512], bf16, tag="abe", name="abe")
                nc.vector.tensor_copy(ab_e[:], acc_e[:])
                ab_o = accp.tile([128, 512], bf16, tag="abo", name="abo")
                nc.gpsimd.tensor_copy(ab_o[:], acc_o[:])
                den_queue.append((g + 3, ab_e, ab_o, ps_ctx, h, b, qt))
        while den_queue:
            pop_den()
        # remaining output-projection groups
        while c_cursor < len(c_groups):
            emit_c_group(drain=True)

    nc.compile()
    return nc


def _get_nc():
    if "nc" not in _NC_CACHE:
        _NC_CACHE["nc"] = _build_program()
    return _NC_CACHE["nc"]


def _rope_tables():
    j = np.arange(0, DH, 2, dtype=np.float32) / np.float32(DH)
    inv_freq = (np.float32(1.0) / (np.float32(ROPE_BASE) ** j)).astype(np.float32)
    t = np.arange(T, dtype=np.float32)
    freqs = np.outer(t, inv_freq).astype(np.float32)   # (T, 64)
    c = np.cos(freqs).astype(np.float32).T             # (64, T)
    s = np.sin(freqs).astype(np.float32).T
    A = np.vstack([c, c]).astype(np.float32)           # (128, T)
    Bt = np.vstack([-s, s]).astype(np.float32)
    return np.ascontiguousarray(A).astype(BF), np.ascontiguousarray(Bt).astype(BF)


def _tri_mask():
    # additive causal mask for a diagonal 128x128 block: key p visible to
    # query j iff p <= j; exp((s - 2e4) * scale) == 0 otherwise
    p = np.arange(128)[:, None]
    j = np.arange(128)[None, :]
    return np.where(p <= j, 0.0, -20000.0).astype(np.float32).astype(BF)


def _make_in_maps(x, Wq, Wk, Wv, Wo):
    xT = np.ascontiguousarray(x.reshape(NT, D).T).astype(BF)
    A, Bt = _rope_tables()
    tri = _tri_mask()

    in_maps = []
    for g in range(8):
        in_maps.append({
            "xT": xT,
            "wq": np.ascontiguousarray(Wq[:, g * FPC:(g + 1) * FPC]).astype(BF),
            "wk": np.ascontiguousarray(Wk[:, g * DH:(g + 1) * DH]).astype(BF),
            "wv": np.ascontiguousarray(Wv[:, g * DH:(g + 1) * DH]).astype(BF),
            "wo": np.ascontiguousarray(Wo[g * FPC:(g + 1) * FPC, :]).astype(BF),
            "ropeA": A,
            "ropeB": Bt,
            "trim": tri,
        })
    return in_maps


def kernel(x, Wq, Wk, Wv, Wo):
    x = np.asarray(x, dtype=np.float32)
    Wq = np.asarray(Wq, dtype=np.float32)
    Wk = np.asarray(Wk, dtype=np.float32)
    Wv = np.asarray(Wv, dtype=np.float32)
    Wo = np.asarray(Wo, dtype=np.float32)

    nc = _get_nc()
    in_maps = _make_in_maps(x, Wq, Wk, Wv, Wo)

    res = run_bass_kernel_spmd(nc, in_maps, list(range(8)))
    acc = res.results[0]["outp"].astype(np.float32)
    for g in range(1, 8):
        acc = acc + res.results[g]["outp"].astype(np.float32)
    # outp blocks: gi = m*8 + n_ -> out[m*128+p, n_*512+c]
    out = acc.reshape(NT // 128, D // 512, 128, 512).transpose(0, 2, 1, 3)
    return np.ascontiguousarray(out.reshape(B, T, D), dtype=np.float32)


# revision 18
# speedup vs baseline: 1.0549x; 1.0507x over previous
"""GroupedQueryAttention TRN2 Bass kernel, sharded over 8 NeuronCores.

Problem (hardcoded): B=2, T=2048, D=4096, 32 Q heads x 128, 8 KV groups x 128,
RoPE (base 5e5), causal, out = ctx @ Wo.

Sharding: core g owns Q heads 4g..4g+3 (Wq columns 512g:512g+512), KV group g
(Wk/Wv columns 128g:128g+128), and Wo rows 512g:512g+512 (row-parallel).
Each core computes a full-shape partial output; host sums the 8 partials.

v5 (over v4): single pool scope with a unified PSUM ring — no pool-close
barrier between projection and attention phases, so the attention seeds
overlap the last projection tile's RoPE tail; softmax denominator reduction
via an all-ones [128,128] stationary matmul that reduces AND broadcasts in
one op (drops the gpsimd broadcast from the critical path); output-projection
evictions moved off the exp-laden ACT queue (DVE/gpsimd rotate); Wo reuses
Wq's SBUF ring slot; startup DMAs issued from four engine queues in
parallel; drain-phase output DMAs split across two queues.
"""
import sys
import numpy as np

for _p in ("/opt/trn_rl_repo", "/root/.axon_site", "/root/.axon_site/_ro/trn_rl_repo"):
    if _p not in sys.path:
        sys.path.append(_p)

from contextlib import ExitStack

import ml_dtypes

import concourse.bass as bass
import concourse.tile as tile
from concourse import bacc, mybir
from concourse.bass_utils import run_bass_kernel_spmd
from concourse.masks import make_identity

B, T, D = 2, 2048, 4096
NH, NKV, DH = 32, 8, 128
HPC = NH // 8          # 4 q heads per core
FPC = HPC * DH         # 512 q features per core
ROPE_BASE = 500000.0
NT = B * T             # 4096 tokens
f32 = mybir.dt.float32
bf16 = mybir.dt.bfloat16
EXP_SCALE = 1.0 / float(np.sqrt(DH))
BF = ml_dtypes.bfloat16

_NC_CACHE = {}


def _build_program():
    nc = bacc.Bacc("TRN2", target_bir_lowering=False, debug=False)

    xT = nc.dram_tensor("xT", [D, NT], bf16, kind="ExternalInput").ap()
    wq = nc.dram_tensor("wq", [D, FPC], bf16, kind="ExternalInput").ap()
    wk = nc.dram_tensor("wk", [D, DH], bf16, kind="ExternalInput").ap()
    wv = nc.dram_tensor("wv", [D, DH], bf16, kind="ExternalInput").ap()
    wo = nc.dram_tensor("wo", [FPC, D], bf16, kind="ExternalInput").ap()
    ropeA = nc.dram_tensor("ropeA", [128, T], bf16, kind="ExternalInput").ap()
    ropeB = nc.dram_tensor("ropeB", [128, T], bf16, kind="ExternalInput").ap()
    trim = nc.dram_tensor("trim", [128, 128], bf16, kind="ExternalInput").ap()
    # output stored as contiguous [128,512] blocks: gi = m*8 + n_
    outp = nc.dram_tensor("outp", [(NT // 128) * (D // 512), 128, 512], bf16,
                          kind="ExternalOutput").ap()

    KC = D // 128  # 32 contraction chunks
    SLAB = 4
    NSLAB = KC // SLAB

    with tile.TileContext(nc) as tc, ExitStack() as s0:
        kvp = s0.enter_context(tc.tile_pool(name="kv", bufs=1))
        # persistent SBUF-resident intermediates
        Qh = [[kvp.tile([128, T], bf16, tag=f"Q{h}b{i}", name=f"Q{h}b{i}")
               for i in range(B)] for h in range(HPC)]
        KTb = [kvp.tile([128, T], bf16, tag=f"KT{i}", name=f"KT{i}") for i in range(B)]
        Vb = [kvp.tile([128, T], bf16, tag=f"V{i}", name=f"V{i}") for i in range(B)]
        Ch = [kvp.tile([128, NT], bf16, tag=f"C{h}", name=f"C{h}") for h in range(HPC)]
        tabA = kvp.tile([128, T], bf16, tag="tabA")
        tabB = kvp.tile([128, T], bf16, tag="tabB")
        tri = kvp.tile([128, 128], bf16, tag="tri")

        wp = s0.enter_context(tc.tile_pool(name="wts", bufs=1))
        wq_sb = wp.tile([128, KC * FPC], bf16, tag="wq")
        wk_sb = wp.tile([128, KC * DH], bf16, tag="wk")
        wv_sb = wp.tile([128, KC * DH], bf16, tag="wv")
        xsp = s0.enter_context(tc.tile_pool(name="xs", bufs=3))
        evp = s0.enter_context(tc.tile_pool(name="ev", bufs=1))
        # unified PSUM ring: tag "a" = 6 banks (projection accumulators /
        # score ring / output-projection / denominator), tag "ctx" = 2 banks
        # (V transposes in phase A, ctx accumulators in phase B)
        psU = s0.enter_context(tc.tile_pool(name="psU", bufs=1, space="PSUM"))
        # phase B pools (allocated up front; no barrier between phases)
        obp = s0.enter_context(tc.tile_pool(name="obp", bufs=6))
        sxp = s0.enter_context(tc.tile_pool(name="sxp", bufs=8))
        smp = s0.enter_context(tc.tile_pool(name="smp", bufs=2))
        accp = s0.enter_context(tc.tile_pool(name="accp", bufs=2))

        def load_w_slab(s, engines=None):
            # one consolidated DMA per weight tensor per 4-chunk slab
            e = engines or (nc.sync, nc.sync, nc.sync)
            e[0].dma_start(
                wq_sb[:, s * SLAB * FPC:(s + 1) * SLAB * FPC]
                .rearrange("p (k c) -> p k c", k=SLAB),
                wq[s * 512:(s + 1) * 512, :]
                .rearrange("(k p) c -> p k c", k=SLAB))
            e[1].dma_start(
                wk_sb[:, s * SLAB * DH:(s + 1) * SLAB * DH]
                .rearrange("p (k c) -> p k c", k=SLAB),
                wk[s * 512:(s + 1) * 512, :]
                .rearrange("(k p) c -> p k c", k=SLAB))
            e[2].dma_start(
                wv_sb[:, s * SLAB * DH:(s + 1) * SLAB * DH]
                .rearrange("p (k c) -> p k c", k=SLAB),
                wv[s * 512:(s + 1) * 512, :]
                .rearrange("(k p) c -> p k c", k=SLAB))

        def load_xsl(n, s):
            xsl = xsp.tile([128, SLAB * 512], bf16, tag="xs", name="xsl")
            nc.sync.dma_start(
                xsl[:].rearrange("p (k c) -> p k c", k=SLAB),
                xT[s * 512:(s + 1) * 512, n * 512:(n + 1) * 512]
                .rearrange("(k p) c -> p k c", k=SLAB))
            return xsl

        # critical-path first: interleave slab-0 weight and x chunks at
        # 128-row granularity, issued from four engine queues in parallel so
        # the sync-queue descriptor rate doesn't gate the pipeline
        xsl0 = xsp.tile([128, SLAB * 512], bf16, tag="xs", name="xsl")
        for k in range(SLAB):
            nc.sync.dma_start(wq_sb[:, k * FPC:(k + 1) * FPC],
                              wq[k * 128:(k + 1) * 128, :])
            nc.scalar.dma_start(xsl0[:, k * 512:(k + 1) * 512],
                                xT[k * 128:(k + 1) * 128, 0:512])
            nc.gpsimd.dma_start(wk_sb[:, k * DH:(k + 1) * DH],
                                wk[k * 128:(k + 1) * 128, :])
            nc.gpsimd.dma_start(wv_sb[:, k * DH:(k + 1) * DH],
                                wv[k * 128:(k + 1) * 128, :])
        load_w_slab(1, engines=(nc.scalar, nc.gpsimd, nc.sync))

        # setup that doesn't gate the first matmuls
        ident_f = kvp.tile([128, 128], f32, tag="ident_f")
        make_identity(nc, ident_f[:])
        ones_t = kvp.tile([128, 128], f32, tag="ones_t")
        nc.vector.memset(ones_t[:], 1.0)
        ones_f = kvp.tile([128, 128], mybir.dt.float32r, tag="ones_f")
        nc.vector.tensor_copy(ones_f[:], ones_t[:])

        # ---------------- Phase A: projections + RoPE -----------------
        def stationary(m, k):
            if m < HPC:
                return wq_sb[:, k * FPC + m * 128: k * FPC + (m + 1) * 128]
            if m == HPC:
                return wk_sb[:, k * DH:(k + 1) * DH]
            return wv_sb[:, k * DH:(k + 1) * DH]

        pending_vt = None

        def flush_vt():
            nonlocal pending_vt
            if pending_vt is None:
                return
            vt_p, n_p = pending_vt
            b_p = n_p // 4
            for i in range(4):
                # f32 transpose into a "ctx" PSUM slot (unused in phase A)
                ptr = psU.tile([128, 512], f32, tag="ctx", bufs=2, name="ptr")
                nc.tensor.transpose(ptr[:, 0:128], vt_p[:, i * 128:(i + 1) * 128],
                                    ident_f[:])
                c_local = 4 * (n_p % 4) + i
                nc.scalar.copy(Vb[b_p][:, c_local * 128:c_local * 128 + 128],
                               ptr[:, 0:128])
            pending_vt = None

        for n in range(NT // 512):
            b, tloc = n // 4, 512 * (n % 4)
            ps = [psU.tile([128, 512], f32, tag="a", bufs=6, name=f"ps{m}")
                  for m in range(6)]
            for s in range(NSLAB):
                if n == 0:
                    xsl = xsl0 if s == 0 else xsl_next
                    if s + 1 < NSLAB:
                        xsl_next = load_xsl(0, s + 1)
                    if s + 2 < NSLAB:
                        load_w_slab(s + 2)
                else:
                    xsl = load_xsl(n, s)
                for m in range(6):
                    for j in range(SLAB):
                        k = s * SLAB + j
                        nc.tensor.matmul(ps[m][:], stationary(m, k),
                                         xsl[:, j * 512:(j + 1) * 512],
                                         start=(k == 0), stop=(k == KC - 1))
                if s == 0:
                    flush_vt()   # prev n-tile's V transposes, PE already warm here
            if n == 0:
                nc.sync.dma_start(tabA[:], ropeA)
                nc.sync.dma_start(tabB[:], ropeB)
            if n == 1:
                nc.sync.dma_start(tri[:], trim)
            # evict: split across ACT and DVE so the PSUM banks drain in
            # parallel
            qes = []
            for m in range(5):
                qe = evp.tile([128, 512], bf16, tag="qe", bufs=6, name=f"qe{m}")
                if m < 3:
                    nc.scalar.copy(qe[:], ps[m][:])
                else:
                    nc.vector.tensor_copy(qe[:], ps[m][:])
                qes.append(qe)
            vt = evp.tile([128, 512], f32, tag="vt", bufs=2, name="vt")
            nc.scalar.copy(vt[:], ps[5][:])
            pending_vt = (vt, n)
            # rope chains on DVE (bf16, 2x rate)
            tA = tabA[:, tloc:tloc + 512]
            tB = tabB[:, tloc:tloc + 512]
            for m in range(5):
                qe = qes[m]
                sw = evp.tile([128, 512], bf16, tag="sw", bufs=1, name="sw")
                nc.vector.tensor_copy(sw[0:64, :], qe[64:128, :])
                nc.vector.tensor_copy(sw[64:128, :], qe[0:64, :])
                mm = evp.tile([128, 512], bf16, tag="mm", bufs=1, name="mm")
                nc.vector.tensor_mul(mm[:], sw[:], tB)
                tt = evp.tile([128, 512], bf16, tag="tt", bufs=1, name="tt")
                nc.vector.tensor_mul(tt[:], qe[:], tA)
                if m < HPC:
                    nc.vector.tensor_add(Qh[m][b][:, tloc:tloc + 512], tt[:], mm[:])
                else:
                    nc.vector.tensor_add(KTb[b][:, tloc:tloc + 512], tt[:], mm[:])
        flush_vt()

        # Wo prefetch into Wq's SBUF ring slot (same shape, bufs=1): the DMA
        # naturally waits for the last Q-projection matmul, no barrier needed.
        wo_sb = wp.tile([128, KC * FPC], bf16, tag="wq", name="wo_sb")
        wo_e = (nc.sync, nc.scalar, nc.gpsimd, nc.sync)
        for h in range(HPC):
            wo_e[h].dma_start(wo_sb[:, h * D:(h + 1) * D], wo[h * 128:(h + 1) * 128, :])

        # ---------------- Phase B: attention -----------------

        # Flat global iteration stream, qt-major so output-projection
        # groups unlock as soon as all 4 heads finish a (b,qt).
        # item = (b, h, qt, kt, q-offset, width, idx-in-qt, n-in-qt)
        items = []
        for b in range(B):
            for qt in range(4):
                for h in range(HPC):
                    plan = [(kt, 0, 512) for kt in range(4 * qt)]
                    plan += [(4 * qt + r, 128 * r, 512 - 128 * r) for r in range(4)]
                    for i, (kt, off, w) in enumerate(plan):
                        items.append((b, h, qt, kt, off, w, i, len(plan)))

        def issue_st(g):
            b, h, qt, kt, off, w, i, npl = items[g]
            ps_st = psU.tile([128, 512], f32, tag="a", bufs=6, name="ps_st")
            nc.tensor.matmul(ps_st[:, 0:w],
                             KTb[b][:, kt * 128:(kt + 1) * 128],
                             Qh[h][b][:, qt * 512 + off: qt * 512 + off + w],
                             start=True, stop=True)
            if kt - 4 * qt >= 0:
                # diagonal block: additive causal mask (-2e4 on the
                # strictly-lower triangle) folded into PSUM on DVE; the
                # depth-3 prefetch hides the extra hop before the exp
                nc.vector.tensor_add(ps_st[:, 0:128], ps_st[:, 0:128], tri[:])
            se = sxp.tile([128, 512], bf16, tag="se", name="se")
            nc.scalar.activation(se[:, 0:w], ps_st[:, 0:w],
                                 mybir.ActivationFunctionType.Exp,
                                 scale=EXP_SCALE)
            return se

        # Phase C output-projection groups, interleaved into the attention
        # stream as PE filler once their tokens' ctx is complete
        c_groups = [(m, n_) for m in range(NT // 128) for n_ in range(D // 512)]
        c_cursor = 0
        c_limit = 0
        C_START_MIN = 48  # don't emit before the Wo DMA has surely landed

        def emit_c_group(drain=False):
            nonlocal c_cursor
            if c_cursor >= len(c_groups):
                return
            m, n_ = c_groups[c_cursor]
            gi = c_cursor
            c_cursor += 1
            pso = psU.tile([128, 512], f32, tag="a", bufs=6, name="pso")
            for hh in range(HPC):
                nc.tensor.matmul(pso[:], Ch[hh][:, m * 128:(m + 1) * 128],
                                 wo_sb[:, hh * D + n_ * 512: hh * D + (n_ + 1) * 512],
                                 start=(hh == 0), stop=(hh == HPC - 1))
            ob = obp.tile([128, 512], bf16, tag="ob", name="ob")
            if drain:
                # endgame: ACT/exp is done; keep evictions on DVE only and
                # spread DMA descriptor issue across two idle queues
                nc.vector.tensor_copy(ob[:], pso[:])
                (nc.sync if gi % 2 else nc.gpsimd).dma_start(outp[gi], ob[:])
            else:
                # alternate eviction engines so neither the exp-laden ACT
                # queue nor the flush-laden DVE queue becomes the bottleneck
                if gi % 2:
                    nc.vector.tensor_copy(ob[:], pso[:])
                else:
                    nc.scalar.copy(ob[:], pso[:])
                nc.sync.dma_start(outp[gi], ob[:])

        # one-time 5-deep seed fills the cold exp chain at attention
        # start; the issue cursor then settles to sustainable depth 4
        se_q = [issue_st(0), issue_st(1), issue_st(2), issue_st(3), issue_st(4)]
        next_issue = 5
        ps_ctx = acc_e = acc_o = None
        den_queue = []  # (ready_g, accb_e, accb_o, ps_ctx, h, b, qt)

        def pop_den():
            # deferred 3 items past the last accumulate; the f32 partial
            # accumulators stream into the PE as float32r (full rate, no
            # casts on the critical chain); all-ones stationary reduces
            # across partitions AND broadcasts to all 128 output partitions,
            # so the normalization multiply can follow immediately
            nonlocal c_limit
            _, a_e, a_o, ps_ctx_p, h_p, b_p, qt_p = den_queue.pop(0)
            den_ps = psU.tile([128, 512], f32, tag="a", bufs=6, name="den")
            nc.tensor.matmul(den_ps[:], ones_f[:], a_e[:],
                             start=True, stop=False, skip_group_check=True)
            nc.tensor.matmul(den_ps[:], ones_f[:], a_o[:],
                             start=False, stop=True, skip_group_check=True)
            rec = smp.tile([128, 512], f32, tag="rec", name="rec")
            nc.vector.reciprocal_approx_fast(out=rec[:], in_=den_ps[:])
            nc.vector.tensor_mul(
                Ch[h_p][:, b_p * T + qt_p * 512: b_p * T + (qt_p + 1) * 512],
                ps_ctx_p[:], rec[:])
            # h==3 with qt-major order: those token blocks are now complete
            # across all heads; their output-projection groups may be emitted
            if h_p == 3:
                c_limit = 8 * (16 * b_p + 4 * (qt_p + 1))

        for g, (b, h, qt, kt, off, w, i, npl) in enumerate(items):
            se_cur = se_q.pop(0)
            if next_issue < len(items) and next_issue <= g + 4:
                se_q.append(issue_st(next_issue))
                next_issue += 1
            if i == 0:
                ps_ctx = psU.tile([128, 512], f32, tag="ctx", bufs=2, name="ps_ctx")
            last = (i == npl - 1)
            nc.tensor.matmul(ps_ctx[:, off:off + w],
                             Vb[b][:, kt * 128:(kt + 1) * 128],
                             se_cur[:, 0:w],
                             start=(i == 0), stop=last,
                             skip_group_check=True)
            # softmax denominator accumulated off the PE, split into two
            # independent chains: even items on DVE, odd items on gpsimd
            # (both SBUF-only, so the Pool engine is legal here)
            if i == 0:
                acc_e = accp.tile([128, 512], mybir.dt.float32r, tag="acce", name="acce")
                nc.vector.tensor_copy(acc_e[:], se_cur[:])
            elif i == 1:
                acc_o = accp.tile([128, 512], mybir.dt.float32r, tag="acco", name="acco")
                nc.gpsimd.tensor_copy(acc_o[:, off:off + w], se_cur[:, 0:w])
                if off:
                    nc.gpsimd.tensor_scalar_mul(acc_o[:, 0:off],
                                                se_cur[:, 0:off], 0.0)
            elif i % 2 == 0:
                nc.vector.tensor_add(acc_e[:, off:off + w], acc_e[:, off:off + w],
                                     se_cur[:, 0:w])
            else:
                nc.gpsimd.tensor_add(acc_o[:, off:off + w], acc_o[:, off:off + w],
                                     se_cur[:, 0:w])
            if den_queue and den_queue[0][0] <= g:
                pop_den()
            if c_cursor < c_limit and g >= C_START_MIN:
                emit_c_group()
                if c_limit - c_cursor >= 8 and c_cursor < c_limit:
                    emit_c_group()
            if last:
                den_queue.append((g + 3, acc_e, acc_o, ps_ctx, h, b, qt))
        while den_queue:
            pop_den()
        # remaining output-projection groups
        while c_cursor < len(c_groups):
            emit_c_group(drain=True)

    nc.compile()
    return nc


def _get_nc():
    if "nc" not in _NC_CACHE:
        _NC_CACHE["nc"] = _build_program()
    return _NC_CACHE["nc"]


def _rope_tables():
    j = np.arange(0, DH, 2, dtype=np.float32) / np.float32(DH)
    inv_freq = (np.float32(1.0) / (np.float32(ROPE_BASE) ** j)).astype(np.float32)
    t = np.arange(T, dtype=np.float32)
    freqs = np.outer(t, inv_freq).astype(np.float32)   # (T, 64)
    c = np.cos(freqs).astype(np.float32).T             # (64, T)
    s = np.sin(freqs).astype(np.float32).T
    A = np.vstack([c, c]).astype(np.float32)           # (128, T)
    Bt = np.vstack([-s, s]).astype(np.float32)
    return np.ascontiguousarray(A).astype(BF), np.ascontiguousarray(Bt).astype(BF)


def _tri_mask():
    # additive causal mask for a diagonal 128x128 block: key p visible to
    # query j iff p <= j; exp((s - 2e4) * scale) == 0 otherwise
    p = np.arange(128)[:, None]
    j = np.arange(128)[None, :]
    return np.where(p <= j, 0.0, -20000.0).astype(np.float32).astype(BF)


def _make_in_maps(x, Wq, Wk, Wv, Wo):
    xT = np.ascontiguousarray(x.reshape(NT, D).T).astype(BF)
    A, Bt = _rope_tables()
    tri = _tri_mask()

    in_maps = []
    for g in range(8):
        in_maps.append({
            "xT": xT,
            "wq": np.ascontiguousarray(Wq[:, g * FPC:(g + 1) * FPC]).astype(BF),
            "wk": np.ascontiguousarray(Wk[:, g * DH:(g + 1) * DH]).astype(BF),
            "wv": np.ascontiguousarray(Wv[:, g * DH:(g + 1) * DH]).astype(BF),
            "wo": np.ascontiguousarray(Wo[g * FPC:(g + 1) * FPC, :]).astype(BF),
            "ropeA": A,
            "ropeB": Bt,
            "trim": tri,
        })
    return in_maps


def kernel(x, Wq, Wk, Wv, Wo):
    x = np.asarray(x, dtype=np.float32)
    Wq = np.asarray(Wq, dtype=np.float32)
    Wk = np.asarray(Wk, dtype=np.float32)
    Wv = np.asarray(Wv, dtype=np.float32)
    Wo = np.asarray(Wo, dtype=np.float32)

    nc = _get_nc()
    in_maps = _make_in_maps(x, Wq, Wk, Wv, Wo)

    res = run_bass_kernel_spmd(nc, in_maps, list(range(8)))
    acc = res.results[0]["outp"].astype(np.float32)
    for g in range(1, 8):
        acc = acc + res.results[g]["outp"].astype(np.float32)
    # outp blocks: gi = m*8 + n_ -> out[m*128+p, n_*512+c]
    out = acc.reshape(NT // 128, D // 512, 128, 512).transpose(0, 2, 1, 3)
    return np.ascontiguousarray(out.reshape(B, T, D), dtype=np.float32)
